# revision 11
# baseline (speedup 1.0000x reference)
"""Causal multi-head attention on 8 TRN2 NeuronCores.

Sharding: core c -> (batch b = c // 2, head-half hh = c % 2).
Each core computes QKV for its 8 heads over the full sequence of its batch,
causal flash attention, and a partial out-projection using its 512 rows of
w_out. The host sums the two partials per batch (the "all-reduce" of the
tensor-parallel out projection).

All matmul operands are bf16 (same PE rate as fp32r, half the DMA/SBUF
traffic; measured end-to-end rel err ~5e-3 vs 2e-2 budget). PSUM stays fp32.

Layouts (per core):
  KT[j]  [128, 2048] bf16  K^T for head pair j (head 2j rows 0:64,
                           head 2j+1 rows 64:128)
  V[t]   [128, 520]  bf16  V token-tile t, 8 heads x (64 cols + ones col)
                           for the softmax denominator ("ones trick");
                           ones cols written on-chip (no descriptor spam)
  QT[j]  [128, 512]  bf16  per-q-chunk Q^T, same head-pair row split as KT;
                           S matmuls contract K=64 on row halves (no
                           zero-padding needed at bf16 rates)

Shapes (hardcoded): B=4, T=2048, D=1024, H=16, HD=64.
"""
import sys

for _p in ('/opt/trn_rl_repo', '/root/.axon_site/_ro/trn_rl_repo'):
    if _p not in sys.path:
        sys.path.insert(0, _p)

import numpy as np

B, T, D = 4, 2048, 1024
H, HD = 16, 64
HPC = H // 2          # heads per core = 8
DPC = HPC * HD        # out-dims per core = 512
N_CORES = 8

_nc_cache = {}


def _build_nc():
    import concourse.bacc as bacc
    import concourse.mybir as mybir
    from concourse.tile import TileContext

    F32 = mybir.dt.float32
    BF16 = mybir.dt.bfloat16
    AF = mybir.ActivationFunctionType
    ALU = mybir.AluOpType

    CH = 512              # phase-1 token chunk (== QC)
    QC = 512              # phase-2 query chunk
    NKB = T // 128        # 16 k-blocks
    NQC = T // QC         # 4 query chunks
    NCH = T // CH         # 4 phase-1 chunks
    NDT = D // 128        # 8 input-dim tiles
    VW = HPC * (HD + 1)   # V tile width = 520

    nc = bacc.Bacc('TRN2', target_bir_lowering=False, debug=False)
    xT_d = nc.dram_tensor('xT', [D, T], BF16, kind='ExternalInput')
    wq_d = nc.dram_tensor('wq', [D, DPC], BF16, kind='ExternalInput')
    wk_d = nc.dram_tensor('wk', [D, DPC], BF16, kind='ExternalInput')
    wv_d = nc.dram_tensor('wv', [D, DPC], BF16, kind='ExternalInput')
    wo_d = nc.dram_tensor('wo', [DPC, D], BF16, kind='ExternalInput')
    po_d = nc.dram_tensor('po', [T, D], BF16, kind='ExternalOutput')

    with nc.allow_low_precision(reason='bf16 matmuls by design'), \
            TileContext(nc) as tc:
        with (
            tc.tile_pool(name='kt', bufs=1) as kt_pool,
            tc.tile_pool(name='vv', bufs=1) as v_pool,
            tc.tile_pool(name='small', bufs=2) as sm_pool,
            tc.tile_pool(name='wq', bufs=1) as wq_pool,
            tc.tile_pool(name='qt', bufs=2) as qt_pool,
            tc.tile_pool(name='xs', bufs=3) as x_pool,
        ):
            WQ = [wq_pool.tile([128, DPC], BF16, tag=f'wq{d}',
                               name=f'wqs{d}') for d in range(NDT)]

            # pre-warm the ACT exp table during phase 1 so the first real
            # exp doesn't pay the ~2.7us table load
            warm = sm_pool.tile([1, 16], F32, tag='warm', bufs=1)
            warm2 = sm_pool.tile([2, 16], F32, tag='warm2', bufs=1)
            nc.vector.memset(warm[:, :], 0.0)
            nc.scalar.activation(warm[:, :], warm[:, :], AF.Exp)
            nc.gpsimd.affine_select(
                out=warm[:, :], in_=warm[:, :], compare_op=ALU.is_ge,
                fill=0.0, base=0, channel_multiplier=-1, pattern=[[1, 16]])
            nc.gpsimd.partition_broadcast(warm2[:, :], warm[:, :])

            KT = [kt_pool.tile([128, T], BF16, tag=f'kt{j}', name=f'kt{j}')
                  for j in range(4)]
            V = [v_pool.tile([128, VW], BF16, tag=f'v{t}', name=f'v{t}')
                 for t in range(NKB)]

            # x chunk tiles: bufs=2 rotation; chunks 2,3 stay resident after
            # phase 1 and feed the phase-2 Q projections (no re-DMA).
            xs_sets = {}

            def alloc_xs(c):
                xs_sets[c] = [x_pool.tile([128, CH], BF16, tag=f'x{d}',
                                          name=f'xs{d}_{c}')
                              for d in range(NDT)]
                return xs_sets[c]

            qt_sets = {}

            def alloc_qt(c):
                qt_sets[c] = [qt_pool.tile([128, QC], BF16, tag=f'qt{j}',
                                           name=f'qt{j}_{c & 1}')
                              for j in range(4)]
                return qt_sets[c]

            # ---------------- Phase 1: K, V (and Q0/Q1) projections --------
            with (
                tc.tile_pool(name='wkv', bufs=1) as w_pool,
                tc.tile_pool(name='ps1', bufs=4, space='PSUM') as ps1,
            ):
                WK = [w_pool.tile([128, DPC], BF16, tag=f'wk{d}',
                                  name=f'wks{d}') for d in range(NDT)]
                WV = [w_pool.tile([128, DPC], BF16, tag=f'wv{d}',
                                  name=f'wvs{d}') for d in range(NDT)]
                xs0 = alloc_xs(0)
                xs1 = alloc_xs(1)
                # startup-critical DMA order: interleave x / WK tiles so the
                # first K matmul can start after ~2 tiles instead of ~4 MB
                for d in range(NDT):
                    nc.sync.dma_start(xs0[d][:, :], xT_d[d*128:(d+1)*128,
                                                         0:CH])
                    nc.sync.dma_start(WK[d][:, :], wk_d[d*128:(d+1)*128, :])
                for d in range(NDT):
                    nc.sync.dma_start(WV[d][:, :], wv_d[d*128:(d+1)*128, :])
                for d in range(NDT):
                    nc.sync.dma_start(xs1[d][:, :],
                                      xT_d[d*128:(d+1)*128, CH:2*CH])
                for d in range(NDT):
                    nc.sync.dma_start(WQ[d][:, :], wq_d[d*128:(d+1)*128, :])
                # later x chunks: emit the loads now (bufs=3 lets chunk 2
                # land while chunks 0/1 are still live; chunk 3's transfer
                # starts as soon as chunk 0's tiles are fully read)
                for c in range(2, NCH):
                    xs = alloc_xs(c)
                    for d in range(NDT):
                        nc.sync.dma_start(
                            xs[d][:, :],
                            xT_d[d*128:(d+1)*128, c*CH:(c+1)*CH])

                # ones columns for the softmax denominator: on-chip memset
                # (a DMA here costs ~16k tiny descriptors and chokes SP)
                for t in range(NKB):
                    vt3 = V[t].rearrange('p (h c) -> p h c', c=HD + 1)
                    nc.gpsimd.memset(vt3[:, :, HD], 1.0)

                for c in range(NCH):
                    xs = xs_sets[c]
                    # K projection, d-outer so compute starts as soon as the
                    # first x/W tiles land
                    ppK = [ps1.tile([128, CH], F32, tag='p1', name=f'ppk{j}')
                           for j in range(4)]
                    for d in range(NDT):
                        for j in range(4):
                            nc.tensor.matmul(
                                ppK[j][:, :],
                                lhsT=WK[d][:, j*128:(j+1)*128],
                                rhs=xs[d][:, :],
                                start=(d == 0), stop=(d == NDT - 1))
                    for j in range(4):
                        nc.vector.tensor_copy(
                            KT[j][:, c*CH:(c+1)*CH], ppK[j][:, :])
                    # V projection: out [128 tok, DPC dout]
                    ppV = [ps1.tile([128, DPC], F32, tag='pv', name=f'ppv{t}')
                           for t in range(CH // 128)]
                    for d in range(NDT):
                        for tt in range(CH // 128):
                            nc.tensor.matmul(
                                ppV[tt][:, :],
                                lhsT=xs[d][:, tt*128:(tt+1)*128],
                                rhs=WV[d][:, :],
                                start=(d == 0), stop=(d == NDT - 1))
                    for tt in range(CH // 128):
                        vt3 = V[c*(CH // 128) + tt].rearrange(
                            'p (h c) -> p h c', c=HD + 1)
                        nc.vector.tensor_copy(
                            vt3[:, :, 0:HD],
                            ppV[tt].rearrange('p (h c) -> p h c', c=HD))
                    # Q^T for the first two attention chunks
                    if c < 2:
                        QTs = alloc_qt(c)
                        ppQ = [ps1.tile([128, QC], F32, tag='p1',
                                        name=f'ppq{j}') for j in range(4)]
                        for d in range(NDT):
                            for j in range(4):
                                nc.tensor.matmul(
                                    ppQ[j][:, :],
                                    lhsT=WQ[d][:, j*128:(j+1)*128],
                                    rhs=xs[d][:, :],
                                    start=(d == 0), stop=(d == NDT - 1))
                        for j in range(4):
                            nc.vector.tensor_copy(QTs[j][:, :], ppQ[j][:, :])

            # ------------- Phase 2 + 3: attention + out-proj -------------
            with (
                tc.tile_pool(name='wo', bufs=1) as wo_pool,
                tc.tile_pool(name='ao', bufs=2) as ao_pool,
                tc.tile_pool(name='pt', bufs=4) as pt_pool,
                tc.tile_pool(name='osb', bufs=2) as osb_pool,
                tc.tile_pool(name='ps_s', bufs=2, space='PSUM') as ps_s,
                tc.tile_pool(name='ps_ot', bufs=4, space='PSUM') as ps_ot,
            ):
                WO = [wo_pool.tile([128, D], BF16, tag=f'wo{d}',
                                   name=f'wos{d}') for d in range(4)]
                for d in range(4):
                    nc.sync.dma_start(WO[d][:, :], wo_d[d*128:(d+1)*128, :])

                for c in range(NQC):
                    q0 = c * QC
                    nkb = (q0 + QC) // 128      # causal k-blocks this chunk
                    QTs = qt_sets.pop(c)
                    if c + 1 < NQC and c >= 1:
                        # Q^T for chunk c+1 (overlaps this chunk's
                        # attention); x comes from the still-resident
                        # phase-1 xs tiles of chunk c+1
                        xs = xs_sets.pop(c + 1)
                        nQT = alloc_qt(c + 1)
                        ppQ = [ps_ot.tile([128, QC], F32, tag='ot',
                                          name=f'p2q{j}') for j in range(4)]
                        for d in range(NDT):
                            for j in range(4):
                                nc.tensor.matmul(
                                    ppQ[j][:, :],
                                    lhsT=WQ[d][:, j*128:(j+1)*128],
                                    rhs=xs[d][:, :],
                                    start=(d == 0), stop=(d == NDT - 1))
                        for j in range(4):
                            nc.vector.tensor_copy(nQT[j][:, :], ppQ[j][:, :])

                    ao = [ao_pool.tile([128, QC], BF16, tag=f'ao{j}',
                                       name=f'ao{j}') for j in range(4)]
                    for j in range(4):            # head pair (2j, 2j+1)
                        h0, h1 = 2*j, 2*j + 1
                        ot0 = ps_ot.tile([HD + 1, QC], F32, tag='ot',
                                         name='ot0')
                        ot1 = ps_ot.tile([HD + 1, QC], F32, tag='ot',
                                         name='ot1')
                        KTe, KTo = KT[j][0:64, :], KT[j][64:128, :]
                        QTe, QTo = QTs[j][0:64, :], QTs[j][64:128, :]
                        pend = None
                        for kbp in range(nkb // 2):
                            ka, kB = 2*kbp, 2*kbp + 1
                            lo_a = max(0, ka*128 - q0)
                            lo_b = max(0, kB*128 - q0)
                            s0 = ps_s.tile([128, 2*QC], F32, tag='s',
                                           name='s0')
                            s1 = ps_s.tile([128, 2*QC], F32, tag='s',
                                           name='s1')
                            pt0 = pt_pool.tile([128, 2*QC], BF16, tag='pt',
                                               name='pt0')
                            pt1 = pt_pool.tile([128, 2*QC], BF16, tag='pt',
                                               name='pt1')
                            # head-even S for both k-blocks (K=64 contraction
                            # on the row halves; no zero padding)
                            nc.tensor.matmul(
                                s0[:, lo_a:QC],
                                lhsT=KTe[:, ka*128:(ka+1)*128],
                                rhs=QTe[:, lo_a:QC],
                                start=True, stop=True)
                            nc.tensor.matmul(
                                s0[:, QC+lo_b:2*QC],
                                lhsT=KTe[:, kB*128:(kB+1)*128],
                                rhs=QTe[:, lo_b:QC],
                                start=True, stop=True)
                            nc.scalar.activation(
                                pt0[:, lo_a:2*QC], s0[:, lo_a:2*QC], AF.Exp)
                            # head-even AV of the previous iteration runs
                            # while head-odd S / exp are in flight
                            if pend is not None:
                                for (pk, pl, pc0), (pp0, _pp1) in pend:
                                    nc.tensor.matmul(
                                        ot0[:, pl:QC],
                                        lhsT=V[pk][:, (HD+1)*h0:
                                                   (HD+1)*(h0+1)],
                                        rhs=pp0[:, pc0+pl:pc0+QC],
                                        start=(pk == 0), stop=False)
                            nc.tensor.matmul(
                                s1[:, lo_a:QC],
                                lhsT=KTo[:, ka*128:(ka+1)*128],
                                rhs=QTo[:, lo_a:QC],
                                start=True, stop=True)
                            nc.tensor.matmul(
                                s1[:, QC+lo_b:2*QC],
                                lhsT=KTo[:, kB*128:(kB+1)*128],
                                rhs=QTo[:, lo_b:QC],
                                start=True, stop=True)
                            nc.scalar.activation(
                                pt1[:, lo_a:2*QC], s1[:, lo_a:2*QC], AF.Exp)
                            if pend is not None:
                                for (pk, pl, pc0), (_pp0, pp1) in pend:
                                    nc.tensor.matmul(
                                        ot1[:, pl:QC],
                                        lhsT=V[pk][:, (HD+1)*h1:
                                                   (HD+1)*(h1+1)],
                                        rhs=pp1[:, pc0+pl:pc0+QC],
                                        start=(pk == 0), stop=False)
                            # causal mask on diagonal blocks (strict upper
                            # triangle of the 128-wide band -> 0)
                            for kx, lox, c0 in ((ka, lo_a, 0),
                                                (kB, lo_b, QC)):
                                if kx*128 >= q0:
                                    for ptx in (pt0, pt1):
                                        nc.gpsimd.affine_select(
                                            out=ptx[:, c0+lox:c0+lox+128],
                                            in_=ptx[:, c0+lox:c0+lox+128],
                                            compare_op=ALU.is_ge, fill=0.0,
                                            base=0, channel_multiplier=-1,
                                            pattern=[[1, 128]])
                            pend = [((ka, lo_a, 0), (pt0, pt1)),
                                    ((kB, lo_b, QC), (pt0, pt1))]
                        for (pk, pl, pc0), (pp0, pp1) in pend:
                            nc.tensor.matmul(
                                ot0[:, pl:QC],
                                lhsT=V[pk][:, (HD+1)*h0:(HD+1)*(h0+1)],
                                rhs=pp0[:, pc0+pl:pc0+QC],
                                start=(pk == 0),
                                stop=(pk == nkb - 1))
                            nc.tensor.matmul(
                                ot1[:, pl:QC],
                                lhsT=V[pk][:, (HD+1)*h1:(HD+1)*(h1+1)],
                                rhs=pp1[:, pc0+pl:pc0+QC],
                                start=(pk == 0),
                                stop=(pk == nkb - 1))
                        # normalize both heads of the pair
                        rp0 = sm_pool.tile([1, QC], F32, tag='rp0', bufs=2)
                        rp1 = sm_pool.tile([1, QC], F32, tag='rp1', bufs=2)
                        din0 = sm_pool.tile([1, QC], F32, tag='din0', bufs=2)
                        din1 = sm_pool.tile([1, QC], F32, tag='din1', bufs=2)
                        nc.vector.tensor_copy(din0[:, :], ot0[HD:HD+1, :])
                        nc.vector.tensor_copy(din1[:, :], ot1[HD:HD+1, :])
                        nc.vector.reciprocal_approx_fast(
                            out=rp0[:, :], in_=din0[:, :])
                        nc.vector.reciprocal_approx_fast(
                            out=rp1[:, :], in_=din1[:, :])
                        rbs0 = sm_pool.tile([HD, QC], F32, tag='rbs0', bufs=2)
                        rbs1 = sm_pool.tile([HD, QC], F32, tag='rbs1', bufs=2)
                        nc.gpsimd.partition_broadcast(rbs0[:, :], rp0[:, :])
                        nc.gpsimd.partition_broadcast(rbs1[:, :], rp1[:, :])
                        nc.vector.tensor_tensor(
                            out=ao[j][0:HD, :], in0=ot0[0:HD, :],
                            in1=rbs0[:, :], op=ALU.mult)
                        nc.vector.tensor_tensor(
                            out=ao[j][HD:128, :], in0=ot1[0:HD, :],
                            in1=rbs1[:, :], op=ALU.mult)
                    # fused partial out-projection for this q-chunk
                    for qt in range(QC // 128):
                        os = osb_pool.tile([128, D], BF16, tag='os',
                                           name='os')
                        for half in range(2):
                            pj = ps_ot.tile([128, 512], F32, tag='ot',
                                            name='pj')
                            for d in range(4):
                                nc.tensor.matmul(
                                    pj[:, :],
                                    lhsT=ao[d][:, qt*128:(qt+1)*128],
                                    rhs=WO[d][:, half*512:(half+1)*512],
                                    start=(d == 0), stop=(d == 3))
                            nc.vector.tensor_copy(
                                os[:, half*512:(half+1)*512], pj[:, :])
                            nc.sync.dma_start(
                                po_d[q0+qt*128:q0+(qt+1)*128,
                                     half*512:(half+1)*512],
                                os[:, half*512:(half+1)*512])

    nc.compile()
    return nc


def _get_nc():
    if 'nc' not in _nc_cache:
        _nc_cache['nc'] = _build_nc()
    return _nc_cache['nc']


def kernel(x, w_qkv, w_out, _profile=False):
    import ml_dtypes
    from concourse.bass_utils import run_bass_kernel_spmd

    BF = ml_dtypes.bfloat16
    x = np.asarray(x, dtype=np.float32)
    w_qkv = np.asarray(w_qkv, dtype=np.float32)
    w_out = np.asarray(w_out, dtype=np.float32)

    nc = _get_nc()

    scale = np.float32(1.0 / np.sqrt(HD))
    in_maps = []
    for c in range(N_CORES):
        b, hh = c // 2, c % 2
        s, e = hh * DPC, (hh + 1) * DPC
        in_maps.append({
            'xT': np.ascontiguousarray(x[b].T).astype(BF),
            'wq': np.ascontiguousarray(w_qkv[:, s:e] * scale).astype(BF),
            'wk': np.ascontiguousarray(w_qkv[:, D+s:D+e]).astype(BF),
            'wv': np.ascontiguousarray(w_qkv[:, 2*D+s:2*D+e]).astype(BF),
            'wo': np.ascontiguousarray(w_out[s:e, :]).astype(BF),
        })

    res = run_bass_kernel_spmd(nc, in_maps, core_ids=list(range(N_CORES)),
                               trace=_profile)
    out = np.empty((B, T, D), np.float32)
    for b in range(B):
        out[b] = (res.results[2*b]['po'].astype(np.float32)
                  + res.results[2*b+1]['po'].astype(np.float32))
    if _profile:
        return out, res
    return out


# revision 12
# speedup vs baseline: 1.0418x; 1.0418x over previous
"""Causal multi-head attention on 8 TRN2 NeuronCores.

Sharding: core c -> (batch b = c // 2, head-half hh = c % 2).
Each core computes QKV for its 8 heads over the full sequence of its batch,
causal flash attention, and a partial out-projection using its 512 rows of
w_out. The host sums the two partials per batch (the "all-reduce" of the
tensor-parallel out projection).

Fully fused single-stream schedule: the attention iterations of q-chunk c
interleave "filler" PE jobs — the K/V/Q projections of chunk c+1 and the
out-projection of chunk c-1 — so the tensor engine never drains at chunk
boundaries and the ACT-bound attention stretches stay packed with PE work.

All matmul operands are bf16 (same PE rate as fp32r, half the DMA/SBUF
traffic; measured end-to-end rel err ~5.6e-3 vs 2e-2 budget). PSUM fp32.

Layouts (per core):
  KT[j]  [128, 2048] bf16  K^T, head pair j (head 2j rows 0:64, 2j+1 64:128)
  V[t]   [128, 520]  bf16  V token-tile t, 8 heads x (64 cols + ones col)
                           for the softmax denominator; ones via memset
  QT[j]  [128, 512]  bf16  per-q-chunk Q^T, same row split; S matmuls
                           contract K=64 on the row halves

Shapes (hardcoded): B=4, T=2048, D=1024, H=16, HD=64.
"""
import sys

for _p in ('/opt/trn_rl_repo', '/root/.axon_site/_ro/trn_rl_repo'):
    if _p not in sys.path:
        sys.path.insert(0, _p)

import numpy as np

B, T, D = 4, 2048, 1024
H, HD = 16, 64
HPC = H // 2          # heads per core = 8
DPC = HPC * HD        # out-dims per core = 512
N_CORES = 8

_nc_cache = {}


def _build_nc():
    import concourse.bacc as bacc
    import concourse.mybir as mybir
    from concourse.tile import TileContext

    F32 = mybir.dt.float32
    BF16 = mybir.dt.bfloat16
    AF = mybir.ActivationFunctionType
    ALU = mybir.AluOpType

    CH = 512              # token chunk (== QC)
    QC = 512              # query chunk
    NKB = T // 128        # 16 k-blocks
    NQC = T // QC         # 4 query chunks
    NDT = D // 128        # 8 input-dim tiles
    VW = HPC * (HD + 1)   # V tile width = 520

    nc = bacc.Bacc('TRN2', target_bir_lowering=False, debug=False)
    xT_d = nc.dram_tensor('xT', [D, T], BF16, kind='ExternalInput')
    wq_d = nc.dram_tensor('wq', [D, DPC], BF16, kind='ExternalInput')
    wk_d = nc.dram_tensor('wk', [D, DPC], BF16, kind='ExternalInput')
    wv_d = nc.dram_tensor('wv', [D, DPC], BF16, kind='ExternalInput')
    wo_d = nc.dram_tensor('wo', [DPC, D], BF16, kind='ExternalInput')
    po_d = nc.dram_tensor('po', [T, D], BF16, kind='ExternalOutput')

    with nc.allow_low_precision(reason='bf16 matmuls by design'), \
            TileContext(nc) as tc:
        with (
            tc.tile_pool(name='kt', bufs=1) as kt_pool,
            tc.tile_pool(name='vv', bufs=1) as v_pool,
            tc.tile_pool(name='small', bufs=2) as sm_pool,
            tc.tile_pool(name='wgt', bufs=1) as w_pool,
            tc.tile_pool(name='qt', bufs=2) as qt_pool,
            tc.tile_pool(name='xs', bufs=3) as x_pool,
            tc.tile_pool(name='ao', bufs=2) as ao_pool,
            tc.tile_pool(name='pt', bufs=4) as pt_pool,
            tc.tile_pool(name='osb', bufs=4) as osb_pool,
            tc.tile_pool(name='ps_s', bufs=2, space='PSUM') as ps_s,
            tc.tile_pool(name='ps_ot', bufs=2, space='PSUM') as ps_ot,
            tc.tile_pool(name='ps_pp', bufs=2, space='PSUM') as ps_pp,
        ):
            WQ = [w_pool.tile([128, DPC], BF16, tag=f'wq{d}',
                              name=f'wqs{d}') for d in range(NDT)]
            WK = [w_pool.tile([128, DPC], BF16, tag=f'wk{d}',
                              name=f'wks{d}') for d in range(NDT)]
            WV = [w_pool.tile([128, DPC], BF16, tag=f'wv{d}',
                              name=f'wvs{d}') for d in range(NDT)]
            WO = [w_pool.tile([128, D], BF16, tag=f'wo{d}',
                              name=f'wos{d}') for d in range(4)]

            # pre-warm the ACT exp table so the first real exp doesn't pay
            # the ~2.7us table load
            warm = sm_pool.tile([1, 16], F32, tag='warm', bufs=1)
            warm2 = sm_pool.tile([2, 16], F32, tag='warm2', bufs=1)
            nc.vector.memset(warm[:, :], 0.0)
            nc.scalar.activation(warm[:, :], warm[:, :], AF.Exp)
            nc.gpsimd.affine_select(
                out=warm[:, :], in_=warm[:, :], compare_op=ALU.is_ge,
                fill=0.0, base=0, channel_multiplier=-1, pattern=[[1, 16]])
            nc.gpsimd.partition_broadcast(warm2[:, :], warm[:, :])

            KT = [kt_pool.tile([128, T], BF16, tag=f'kt{j}', name=f'kt{j}')
                  for j in range(4)]
            V = [v_pool.tile([128, VW], BF16, tag=f'v{t}', name=f'v{t}')
                 for t in range(NKB)]

            xs_sets = {}
            for c in range(NQC):
                xs_sets[c] = [x_pool.tile([128, CH], BF16, tag=f'x{d}',
                                          name=f'xs{d}_{c}')
                              for d in range(NDT)]
            qt_sets = {}

            def alloc_qt(c):
                qt_sets[c] = [qt_pool.tile([128, QC], BF16, tag=f'qt{j}',
                                           name=f'qt{j}_{c & 1}')
                              for j in range(4)]
                return qt_sets[c]

            # DMA emission order is startup-critical: interleave chunk-0 x
            # with WK so the first matmul starts after ~2 tiles
            for d in range(NDT):
                nc.sync.dma_start(xs_sets[0][d][:, :],
                                  xT_d[d*128:(d+1)*128, 0:CH])
                nc.sync.dma_start(WK[d][:, :], wk_d[d*128:(d+1)*128, :])
            for d in range(NDT):
                nc.sync.dma_start(WV[d][:, :], wv_d[d*128:(d+1)*128, :])
            for d in range(NDT):
                nc.sync.dma_start(xs_sets[1][d][:, :],
                                  xT_d[d*128:(d+1)*128, CH:2*CH])
            for d in range(NDT):
                nc.sync.dma_start(WQ[d][:, :], wq_d[d*128:(d+1)*128, :])
            for d in range(4):
                nc.sync.dma_start(WO[d][:, :], wo_d[d*128:(d+1)*128, :])
            for c in range(2, NQC):
                for d in range(NDT):
                    nc.sync.dma_start(
                        xs_sets[c][d][:, :],
                        xT_d[d*128:(d+1)*128, c*CH:(c+1)*CH])

            # softmax-denominator ones columns: on-chip (no descriptor spam)
            for t in range(NKB):
                vt3 = V[t].rearrange('p (h c) -> p h c', c=HD + 1)
                nc.gpsimd.memset(vt3[:, :, HD], 1.0)

            def proj_jobs(c):
                """K/V/Q projection of chunk c as 12 single-psum-tile jobs."""
                xs = xs_sets[c]
                QTs = alloc_qt(c)
                jobs = []
                for j in range(4):
                    def kj(j=j, xs=xs, c=c):
                        pp = ps_pp.tile([128, CH], F32, tag='pp',
                                        name=f'ppk{j}')
                        for d in range(NDT):
                            nc.tensor.matmul(
                                pp[:, :],
                                lhsT=WK[d][:, j*128:(j+1)*128],
                                rhs=xs[d][:, :],
                                start=(d == 0), stop=(d == NDT - 1))
                        nc.vector.tensor_copy(
                            KT[j][:, c*CH:(c+1)*CH], pp[:, :])
                    jobs.append(kj)
                for tt in range(4):
                    def vj(tt=tt, xs=xs, c=c):
                        pp = ps_pp.tile([128, DPC], F32, tag='pp',
                                        name=f'ppv{tt}')
                        for d in range(NDT):
                            nc.tensor.matmul(
                                pp[:, :],
                                lhsT=xs[d][:, tt*128:(tt+1)*128],
                                rhs=WV[d][:, :],
                                start=(d == 0), stop=(d == NDT - 1))
                        vt3 = V[c*4 + tt].rearrange('p (h c) -> p h c',
                                                    c=HD + 1)
                        nc.vector.tensor_copy(
                            vt3[:, :, 0:HD],
                            pp.rearrange('p (h c) -> p h c', c=HD))
                    jobs.append(vj)
                for j in range(4):
                    def qj(j=j, xs=xs, QTs=QTs):
                        pp = ps_pp.tile([128, QC], F32, tag='pp',
                                        name=f'ppq{j}')
                        for d in range(NDT):
                            nc.tensor.matmul(
                                pp[:, :],
                                lhsT=WQ[d][:, j*128:(j+1)*128],
                                rhs=xs[d][:, :],
                                start=(d == 0), stop=(d == NDT - 1))
                        nc.vector.tensor_copy(QTs[j][:, :], pp[:, :])
                    jobs.append(qj)
                return jobs

            def outproj_jobs(c, ao):
                """Partial out-projection of chunk c as 8 jobs."""
                q0 = c * QC
                jobs = []
                for qt in range(4):
                    for half in range(2):
                        def oj(qt=qt, half=half, ao=ao, q0=q0):
                            pj = ps_pp.tile([128, 512], F32, tag='pp',
                                            name='pj')
                            for dd in range(4):
                                nc.tensor.matmul(
                                    pj[:, :],
                                    lhsT=ao[dd][:, qt*128:(qt+1)*128],
                                    rhs=WO[dd][:, half*512:(half+1)*512],
                                    start=(dd == 0), stop=(dd == 3))
                            os = osb_pool.tile([128, 512], BF16, tag='os',
                                               name='os')
                            nc.vector.tensor_copy(os[:, :], pj[:, :])
                            nc.sync.dma_start(
                                po_d[q0+qt*128:q0+(qt+1)*128,
                                     half*512:(half+1)*512], os[:, :])
                        jobs.append(oj)
                return jobs

            # chunk-0 projections run standalone (nothing to overlap yet)
            for job in proj_jobs(0):
                job()

            prev_outproj = []
            for c in range(NQC):
                q0 = c * QC
                nkb = (q0 + QC) // 128      # causal k-blocks this chunk
                QTs = qt_sets.pop(c)
                filler = (proj_jobs(c + 1) if c + 1 < NQC else [])
                filler += prev_outproj
                nslots = 4 * (nkb // 2)
                emitted = 0
                it = 0

                ao = [ao_pool.tile([128, QC], BF16, tag=f'ao{j}',
                                   name=f'ao{j}') for j in range(4)]
                for j in range(4):            # head pair (2j, 2j+1)
                    h0, h1 = 2*j, 2*j + 1
                    ot0 = ps_ot.tile([HD + 1, QC], F32, tag='ot',
                                     name='ot0')
                    ot1 = ps_ot.tile([HD + 1, QC], F32, tag='ot',
                                     name='ot1')
                    KTe, KTo = KT[j][0:64, :], KT[j][64:128, :]
                    QTe, QTo = QTs[j][0:64, :], QTs[j][64:128, :]
                    pend = None
                    for kbp in range(nkb // 2):
                        ka, kB = 2*kbp, 2*kbp + 1
                        lo_a = max(0, ka*128 - q0)
                        lo_b = max(0, kB*128 - q0)
                        s0 = ps_s.tile([128, 2*QC], F32, tag='s', name='s0')
                        s1 = ps_s.tile([128, 2*QC], F32, tag='s', name='s1')
                        pt0 = pt_pool.tile([128, 2*QC], BF16, tag='pt',
                                           name='pt0')
                        pt1 = pt_pool.tile([128, 2*QC], BF16, tag='pt',
                                           name='pt1')
                        nc.tensor.matmul(
                            s0[:, lo_a:QC],
                            lhsT=KTe[:, ka*128:(ka+1)*128],
                            rhs=QTe[:, lo_a:QC],
                            start=True, stop=True)
                        nc.tensor.matmul(
                            s0[:, QC+lo_b:2*QC],
                            lhsT=KTe[:, kB*128:(kB+1)*128],
                            rhs=QTe[:, lo_b:QC],
                            start=True, stop=True)
                        nc.scalar.activation(
                            pt0[:, lo_a:2*QC], s0[:, lo_a:2*QC], AF.Exp)
                        if pend is not None:
                            for (pk, pl, pc0), (pp0, _pp1) in pend:
                                nc.tensor.matmul(
                                    ot0[:, pl:QC],
                                    lhsT=V[pk][:, (HD+1)*h0:(HD+1)*(h0+1)],
                                    rhs=pp0[:, pc0+pl:pc0+QC],
                                    start=(pk == 0), stop=False)
                        nc.tensor.matmul(
                            s1[:, lo_a:QC],
                            lhsT=KTo[:, ka*128:(ka+1)*128],
                            rhs=QTo[:, lo_a:QC],
                            start=True, stop=True)
                        nc.tensor.matmul(
                            s1[:, QC+lo_b:2*QC],
                            lhsT=KTo[:, kB*128:(kB+1)*128],
                            rhs=QTo[:, lo_b:QC],
                            start=True, stop=True)
                        nc.scalar.activation(
                            pt1[:, lo_a:2*QC], s1[:, lo_a:2*QC], AF.Exp)
                        if pend is not None:
                            for (pk, pl, pc0), (_pp0, pp1) in pend:
                                nc.tensor.matmul(
                                    ot1[:, pl:QC],
                                    lhsT=V[pk][:, (HD+1)*h1:(HD+1)*(h1+1)],
                                    rhs=pp1[:, pc0+pl:pc0+QC],
                                    start=(pk == 0), stop=False)
                        for kx, lox, c0 in ((ka, lo_a, 0), (kB, lo_b, QC)):
                            if kx*128 >= q0:   # causal mask on diag band
                                for ptx in (pt0, pt1):
                                    nc.gpsimd.affine_select(
                                        out=ptx[:, c0+lox:c0+lox+128],
                                        in_=ptx[:, c0+lox:c0+lox+128],
                                        compare_op=ALU.is_ge, fill=0.0,
                                        base=0, channel_multiplier=-1,
                                        pattern=[[1, 128]])
                        pend = [((ka, lo_a, 0), (pt0, pt1)),
                                ((kB, lo_b, QC), (pt0, pt1))]
                        # interleave filler PE work (next chunk's
                        # projections, previous chunk's out-projection)
                        it += 1
                        want = (len(filler) * it + nslots - 1) // nslots
                        while emitted < want:
                            filler[emitted]()
                            emitted += 1
                    for (pk, pl, pc0), (pp0, pp1) in pend:
                        nc.tensor.matmul(
                            ot0[:, pl:QC],
                            lhsT=V[pk][:, (HD+1)*h0:(HD+1)*(h0+1)],
                            rhs=pp0[:, pc0+pl:pc0+QC],
                            start=(pk == 0), stop=(pk == nkb - 1))
                        nc.tensor.matmul(
                            ot1[:, pl:QC],
                            lhsT=V[pk][:, (HD+1)*h1:(HD+1)*(h1+1)],
                            rhs=pp1[:, pc0+pl:pc0+QC],
                            start=(pk == 0), stop=(pk == nkb - 1))
                    # normalize both heads of the pair
                    rp0 = sm_pool.tile([1, QC], F32, tag='rp0', bufs=2)
                    rp1 = sm_pool.tile([1, QC], F32, tag='rp1', bufs=2)
                    din0 = sm_pool.tile([1, QC], F32, tag='din0', bufs=2)
                    din1 = sm_pool.tile([1, QC], F32, tag='din1', bufs=2)
                    nc.vector.tensor_copy(din0[:, :], ot0[HD:HD+1, :])
                    nc.vector.tensor_copy(din1[:, :], ot1[HD:HD+1, :])
                    nc.vector.reciprocal_approx_fast(
                        out=rp0[:, :], in_=din0[:, :])
                    nc.vector.reciprocal_approx_fast(
                        out=rp1[:, :], in_=din1[:, :])
                    rbs0 = sm_pool.tile([HD, QC], F32, tag='rbs0', bufs=2)
                    rbs1 = sm_pool.tile([HD, QC], F32, tag='rbs1', bufs=2)
                    nc.gpsimd.partition_broadcast(rbs0[:, :], rp0[:, :])
                    nc.gpsimd.partition_broadcast(rbs1[:, :], rp1[:, :])
                    nc.vector.tensor_tensor(
                        out=ao[j][0:HD, :], in0=ot0[0:HD, :],
                        in1=rbs0[:, :], op=ALU.mult)
                    nc.vector.tensor_tensor(
                        out=ao[j][HD:128, :], in0=ot1[0:HD, :],
                        in1=rbs1[:, :], op=ALU.mult)
                while emitted < len(filler):
                    filler[emitted]()
                    emitted += 1
                prev_outproj = outproj_jobs(c, ao)

            for job in prev_outproj:    # chunk 3's out-projection
                job()

    nc.compile()
    return nc


def _get_nc():
    if 'nc' not in _nc_cache:
        _nc_cache['nc'] = _build_nc()
    return _nc_cache['nc']


def kernel(x, w_qkv, w_out, _profile=False):
    import ml_dtypes
    from concourse.bass_utils import run_bass_kernel_spmd

    BF = ml_dtypes.bfloat16
    x = np.asarray(x, dtype=np.float32)
    w_qkv = np.asarray(w_qkv, dtype=np.float32)
    w_out = np.asarray(w_out, dtype=np.float32)

    nc = _get_nc()

    scale = np.float32(1.0 / np.sqrt(HD))
    in_maps = []
    for c in range(N_CORES):
        b, hh = c // 2, c % 2
        s, e = hh * DPC, (hh + 1) * DPC
        in_maps.append({
            'xT': np.ascontiguousarray(x[b].T).astype(BF),
            'wq': np.ascontiguousarray(w_qkv[:, s:e] * scale).astype(BF),
            'wk': np.ascontiguousarray(w_qkv[:, D+s:D+e]).astype(BF),
            'wv': np.ascontiguousarray(w_qkv[:, 2*D+s:2*D+e]).astype(BF),
            'wo': np.ascontiguousarray(w_out[s:e, :]).astype(BF),
        })

    res = run_bass_kernel_spmd(nc, in_maps, core_ids=list(range(N_CORES)),
                               trace=_profile)
    out = np.empty((B, T, D), np.float32)
    for b in range(B):
        out[b] = (res.results[2*b]['po'].astype(np.float32)
                  + res.results[2*b+1]['po'].astype(np.float32))
    if _profile:
        return out, res
    return out
